# revision 11
# baseline (speedup 1.0000x reference)
"""Trainium2 Bass kernel for nn_Decoder (dense transformer decoder layer).

Strategy: pure data-parallel over batch B=256 across 8 NeuronCores (32
batches/core).  Each core runs the full decoder layer on its shard; no
collectives.

Per-core dataflow (per batch sample b, T=200 tokens, E=584 features):
  - residual stream x kept TOKEN-major ([token_partitions, features]) in fp32
  - LayerNorm computed natively token-major (stats along free dim)
  - LN output h transposed on the PE (via identity matmul) to FEATURE-major
    bf16 [E, T] which feeds all weight matmuls (contraction dim = partitions)
  - attention scores computed TRANSPOSED: S^T[s,t] = k_h^T-slices @ q_h
    (lhsT=k, rhs=q), exp on the scalar engine (scale=E^-0.5 fused), softmax
    denominator Z via ones-vector matmul, 1/Z applied at the AV-output evict
    using a DMA partition-broadcast of the reciprocal row
  - AV output per head o_h [HS=73, T] feature-major; attn projection
    accumulates over heads (K-pieces of 73) and produces TOKEN-major output
    (lhsT = o_h with free dim = tokens) so the residual add is native
  - FFN: w1 feature-major out (Relu+bias fused at evict), w2 token-major out
  - all matmul operands bf16 (fp32 PSUM accumulation); residuals fp32

LayerNorm weights/biases are folded on the host into the adjacent projection
weights (exact for ln_w; ln_b folds require the resulting qkv/ffn biases to be
zero, which holds for this problem's inputs and is asserted).
"""

import sys

sys.path.insert(0, "/opt/trn_rl_repo")

from contextlib import ExitStack

import numpy as np
import ml_dtypes

import concourse.bass as bass
import concourse.bacc as bacc
import concourse.mybir as mybir
import concourse.tile as tile
from concourse.bass_utils import run_bass_kernel_spmd

F32 = mybir.dt.float32
BF16 = mybir.dt.bfloat16
BF16NP = ml_dtypes.bfloat16

B, T, E, H = 256, 200, 584, 8
HS = E // H  # 73
FF = 4 * E  # 2336
NCORES = 8
BL = B // NCORES  # 32
SCALE = float(E) ** -0.5
EPS = 1e-5

# tile decompositions
TS = [(0, 128), (1, 72)]  # token tiles (T=200)
EB = [128, 128, 128, 128, 72]  # E=584 partition blocks
EK = 5
FFB = [128] * 18 + [32]  # FF=2336 partition blocks
FFK = 19
NSPL = [0, 292]  # free-dim split of an E-sized matmul output (<=512 psum)


def _head_pieces(h):
    """Split head h's feature rows [73h, 73h+73) at 128-partition block
    boundaries -> list of (block, r0, r1)."""
    g0, g1 = HS * h, HS * h + HS
    out = []
    g = g0
    while g < g1:
        kb, r0 = divmod(g, 128)
        r1 = min(128, r0 + (g1 - g))
        out.append((kb, r0, r1))
        g += r1 - r0
    return out


def build_nc(bl=BL):
    """Build the single-core Bass program processing `bl` batch samples."""
    nc = bacc.Bacc(None, target_bir_lowering=False, debug=False)

    idx_d = nc.dram_tensor("idx", [bl, T, E], F32, kind="ExternalInput")
    mem_d = nc.dram_tensor("mem", [bl, T, E], BF16, kind="ExternalInput")
    w_names = ["wq_sa", "wk_sa", "wv_sa", "wq_ca", "wk_ca", "wv_ca"]
    w_d = {n: nc.dram_tensor(n, [128, EK, E], BF16, kind="ExternalInput") for n in w_names}
    wp_sa_d = nc.dram_tensor("wp_sa", [128, H, E], BF16, kind="ExternalInput")
    wp_ca_d = nc.dram_tensor("wp_ca", [128, H, E], BF16, kind="ExternalInput")
    w1_d = nc.dram_tensor("w1", [128, EK, FF], BF16, kind="ExternalInput")
    w2_d = nc.dram_tensor("w2", [128, FFK, E], BF16, kind="ExternalInput")
    b1_d = nc.dram_tensor("b1", [128, FFK], F32, kind="ExternalInput")
    ident_d = nc.dram_tensor("ident", [128, 128], BF16, kind="ExternalInput")
    mask_d = nc.dram_tensor("mask", [128, 128], BF16, kind="ExternalInput")
    out_d = nc.dram_tensor("out", [bl, T, E], F32, kind="ExternalOutput")

    with tile.TileContext(nc) as tc, ExitStack() as ctx:
        wpool = ctx.enter_context(tc.tile_pool(name="wpool", bufs=1))
        w_sb = {}
        for n in w_names:
            w_sb[n] = wpool.tile([128, EK, E], BF16, name=n + "_sb")
            nc.sync.dma_start(w_sb[n][:], w_d[n][:])
        wp_sa_sb = wpool.tile([128, H, E], BF16, name="wp_sa_sb")
        nc.sync.dma_start(wp_sa_sb[:], wp_sa_d[:])
        wp_ca_sb = wpool.tile([128, H, E], BF16, name="wp_ca_sb")
        nc.sync.dma_start(wp_ca_sb[:], wp_ca_d[:])
        w1_sb = wpool.tile([128, EK, FF], BF16, name="w1_sb")
        nc.sync.dma_start(w1_sb[:], w1_d[:])
        w2_sb = wpool.tile([128, FFK, E], BF16, name="w2_sb")
        nc.sync.dma_start(w2_sb[:], w2_d[:])
        b1_sb = wpool.tile([128, FFK], F32, name="b1_sb")
        nc.sync.dma_start(b1_sb[:], b1_d[:])
        ident_sb = wpool.tile([128, 128], BF16, name="ident_sb")
        nc.sync.dma_start(ident_sb[:], ident_d[:])
        mask_sb = wpool.tile([128, 128], BF16, name="mask_sb")
        nc.sync.dma_start(mask_sb[:], mask_d[:])
        ones_sb = wpool.tile([128, 1], BF16, name="ones_sb")
        nc.vector.memset(ones_sb[:], 1.0)
        eps_sb = wpool.tile([128, 1], F32, name="eps_sb")
        nc.vector.memset(eps_sb[:], EPS)

        resid = ctx.enter_context(tc.tile_pool(name="resid", bufs=2))
        work = ctx.enter_context(tc.tile_pool(name="work", bufs=2))
        stat = ctx.enter_context(tc.tile_pool(name="stat", bufs=2))
        scr = ctx.enter_context(tc.tile_pool(name="scr", bufs=2))
        opool = ctx.enter_context(tc.tile_pool(name="opool", bufs=2))
        ps_mm = ctx.enter_context(tc.tile_pool(name="ps_mm", bufs=3, space="PSUM"))
        ps_s = ctx.enter_context(tc.tile_pool(name="ps_s", bufs=2, space="PSUM"))
        ps_z = ctx.enter_context(tc.tile_pool(name="ps_z", bufs=1, space="PSUM"))
        ps_o = ctx.enter_context(tc.tile_pool(name="ps_o", bufs=1, space="PSUM"))
        ps_tp = ctx.enter_context(tc.tile_pool(name="ps_tp", bufs=1, space="PSUM"))
        dpool = ctx.enter_context(tc.tile_pool(name="dpool", bufs=4, space="DRAM"))

        def layernorm(x_t, name):
            """x_t [128,2,E] f32 -> h_tok [128,2,E] bf16, normalized (no w/b)."""
            h_tok = work.tile([128, 2, E], BF16, name=name, tag="htok")
            for tt, tsz in TS:
                xs = x_t[0:tsz, tt, :]
                s1 = stat.tile([128, 1], F32, name=name + f"_s1_{tt}", tag="s1")
                s2 = stat.tile([128, 1], F32, name=name + f"_s2_{tt}", tag="s2")
                sc1 = scr.tile([128, E], F32, name=name + f"_sc1_{tt}", tag="sc1", bufs=1)
                sc2 = scr.tile([128, E], F32, name=name + f"_sc2_{tt}", tag="sc2", bufs=1)
                nc.scalar.activation(
                    sc1[0:tsz, :], xs, mybir.ActivationFunctionType.Copy,
                    accum_out=s1[0:tsz, :])
                nc.scalar.activation(
                    sc2[0:tsz, :], xs, mybir.ActivationFunctionType.Square,
                    accum_out=s2[0:tsz, :])
                negm = stat.tile([128, 1], F32, name=name + f"_nm_{tt}", tag="nm")
                nc.scalar.mul(negm[0:tsz, :], s1[0:tsz, :], -1.0 / E)
                m2 = stat.tile([128, 1], F32, name=name + f"_m2_{tt}", tag="m2")
                nc.vector.tensor_mul(m2[0:tsz, :], negm[0:tsz, :], negm[0:tsz, :])
                ss = stat.tile([128, 1], F32, name=name + f"_ss_{tt}", tag="ss")
                nc.scalar.mul(ss[0:tsz, :], s2[0:tsz, :], 1.0 / E)
                var = stat.tile([128, 1], F32, name=name + f"_var_{tt}", tag="var")
                nc.vector.tensor_sub(var[0:tsz, :], ss[0:tsz, :], m2[0:tsz, :])
                sd = stat.tile([128, 1], F32, name=name + f"_sd_{tt}", tag="sd")
                nc.scalar.activation(
                    sd[0:tsz, :], var[0:tsz, :],
                    mybir.ActivationFunctionType.Sqrt, bias=eps_sb[0:tsz, :])
                r = stat.tile([128, 1], F32, name=name + f"_r_{tt}", tag="r")
                nc.vector.reciprocal(r[0:tsz, :], sd[0:tsz, :])
                nmr = stat.tile([128, 1], F32, name=name + f"_nmr_{tt}", tag="nmr")
                nc.vector.tensor_mul(nmr[0:tsz, :], negm[0:tsz, :], r[0:tsz, :])
                nc.scalar.activation(
                    h_tok[0:tsz, tt, :], xs,
                    mybir.ActivationFunctionType.Identity,
                    bias=nmr[0:tsz, :], scale=r[0:tsz, :])
            return h_tok

        def to_fm(src_tok, name):
            """[128,2,E] bf16 token-major -> [128,EK,T] bf16 feature-major."""
            dst = work.tile([128, EK, T], BF16, name=name, tag="hfm")
            for tt, tsz in TS:
                for eb in range(EK):
                    esz = EB[eb]
                    ps = ps_tp.tile([128, 128], BF16, name=f"{name}_tp", tag="tp")
                    nc.tensor.transpose(
                        ps[0:esz, 0:tsz],
                        src_tok[0:tsz, tt, eb * 128:eb * 128 + esz],
                        ident_sb[0:tsz, 0:tsz])
                    nc.any.tensor_copy(
                        dst[0:esz, eb, tt * 128:tt * 128 + tsz], ps[0:esz, 0:tsz])
            return dst

        def proj_heads(w, src_fm, name):
            """q/k projection, per-head aligned output [HS, H, T] bf16
            (matmul operands must start at partition 0/32/64, so each head's
            73 rows get their own partition-base-0 slot)."""
            dst = work.tile([HS, H, T], BF16, name=name, tag=name[:1])
            for h in range(H):
                ps = ps_mm.tile([128, T], F32, name=f"{name}_ps", tag="mm")
                for k in range(EK):
                    ksz = EB[k]
                    nc.tensor.matmul(
                        ps[0:HS, :], w[0:ksz, k, HS * h:HS * h + HS],
                        src_fm[0:ksz, k, :], start=(k == 0), stop=(k == EK - 1))
                nc.any.tensor_copy(dst[:, h, :], ps[0:HS, :])
            return dst

        def proj_tok(w, src_fm, name):
            """v projection: token-major out [128,2,E] bf16."""
            dst = work.tile([128, 2, E], BF16, name=name, tag="vtok")
            for mt, msz in TS:
                for n0 in NSPL:
                    ps = ps_mm.tile([128, 292], F32, name=f"{name}_ps", tag="mm")
                    for k in range(EK):
                        ksz = EB[k]
                        nc.tensor.matmul(
                            ps[0:msz, :],
                            src_fm[0:ksz, k, mt * 128:mt * 128 + msz],
                            w[0:ksz, k, n0:n0 + 292],
                            start=(k == 0), stop=(k == EK - 1))
                    nc.any.tensor_copy(dst[0:msz, mt, n0:n0 + 292], ps[0:msz, :])
            return dst

        def attention(q_fm, k_fm, v_tok, wp_sb, x_in, causal, name, xtag):
            """Full MHA given feature-major q/k, token-major v.  Returns new
            residual (token-major f32): x_out = proj(attn) + x_in."""
            o_sb = []
            for h in range(H):
                expS = opool.tile([128, 2, T], BF16, name=f"{name}_e{h}", tag="expS")
                # scores S^T: s-tile 0 (s=0:128), all t (causal: t>=s region)
                ps0 = ps_s.tile([128, T], F32, name=f"{name}_s0_{h}", tag="s")
                nc.tensor.matmul(
                    ps0[0:128, :], k_fm[:, h, 0:128], q_fm[:, h, :],
                    start=True, stop=True)
                nc.scalar.activation(
                    expS[0:128, 0, :], ps0[0:128, :],
                    mybir.ActivationFunctionType.Exp, scale=SCALE)
                if causal:
                    nc.vector.tensor_mul(
                        expS[0:128, 0, 0:128], expS[0:128, 0, 0:128],
                        mask_sb[0:128, 0:128])
                # s-tile 1 (s=128:200)
                ps1 = ps_s.tile([128, T], F32, name=f"{name}_s1_{h}", tag="s")
                t0 = 128 if causal else 0
                tsz1 = T - t0
                nc.tensor.matmul(
                    ps1[0:72, 0:tsz1], k_fm[:, h, 128:200], q_fm[:, h, t0:T],
                    start=True, stop=True)
                nc.scalar.activation(
                    expS[0:72, 1, t0:T], ps1[0:72, 0:tsz1],
                    mybir.ActivationFunctionType.Exp, scale=SCALE)
                if causal:
                    nc.vector.tensor_mul(
                        expS[0:72, 1, 128:200], expS[0:72, 1, 128:200],
                        mask_sb[0:72, 0:72])
                # softmax denominator Z = column-sum over s
                zps = ps_z.tile([1, T], F32, name=f"{name}_z{h}", tag="z")
                if causal:
                    nc.tensor.matmul(zps[0:1, :], ones_sb[0:128, 0:1],
                                     expS[0:128, 0, :], start=True, stop=False)
                    nc.tensor.matmul(zps[0:1, 128:200], ones_sb[0:72, 0:1],
                                     expS[0:72, 1, 128:200], start=False, stop=True)
                else:
                    nc.tensor.matmul(zps[0:1, :], ones_sb[0:128, 0:1],
                                     expS[0:128, 0, :], start=True, stop=False)
                    nc.tensor.matmul(zps[0:1, :], ones_sb[0:72, 0:1],
                                     expS[0:72, 1, :], start=False, stop=True)
                zr = stat.tile([1, T], F32, name=f"{name}_zr{h}", tag="zr")
                nc.vector.reciprocal(zr[0:1, :], zps[0:1, :])
                zd = dpool.tile([1, T], F32, name=f"{name}_zd{h}", tag="zd")
                nc.sync.dma_start(zd[0:1, :], zr[0:1, :])
                zb = scr.tile([128, T], F32, name=f"{name}_zb{h}", tag="zb")
                nc.sync.dma_start(zb[:, :], zd[0:1, :].to_broadcast([128, T]))
                # attention-weighted values: o_h [HS, T] feature-major
                hs0 = HS * h
                ops = ps_o.tile([HS, T], F32, name=f"{name}_o{h}", tag="o")
                if causal:
                    nc.tensor.matmul(ops[:, 0:128], v_tok[0:128, 0, hs0:hs0 + HS],
                                     expS[0:128, 0, 0:128], start=True, stop=True)
                    nc.tensor.matmul(ops[:, 128:200], v_tok[0:128, 0, hs0:hs0 + HS],
                                     expS[0:128, 0, 128:200], start=True, stop=False)
                    nc.tensor.matmul(ops[:, 128:200], v_tok[0:72, 1, hs0:hs0 + HS],
                                     expS[0:72, 1, 128:200], start=False, stop=True)
                else:
                    nc.tensor.matmul(ops[:, :], v_tok[0:128, 0, hs0:hs0 + HS],
                                     expS[0:128, 0, :], start=True, stop=False)
                    nc.tensor.matmul(ops[:, :], v_tok[0:72, 1, hs0:hs0 + HS],
                                     expS[0:72, 1, :], start=False, stop=True)
                osb = opool.tile([HS, T], BF16, name=f"{name}_ob{h}", tag=f"o{h}")
                nc.vector.tensor_mul(osb[:, :], ops[:, :], zb[0:HS, :])
                o_sb.append(osb)
            # projection (accumulate over heads) + residual, token-major out
            x_out = resid.tile([128, 2, E], F32, name=f"{name}_xo", tag=xtag)
            for mt, msz in TS:
                for n0 in NSPL:
                    ps = ps_mm.tile([128, 292], F32, name=f"{name}_pj", tag="mm")
                    for h in range(H):
                        nc.tensor.matmul(
                            ps[0:msz, :],
                            o_sb[h][:, mt * 128:mt * 128 + msz],
                            wp_sb[0:HS, h, n0:n0 + 292],
                            start=(h == 0), stop=(h == H - 1))
                    nc.vector.tensor_add(
                        x_out[0:msz, mt, n0:n0 + 292], ps[0:msz, :],
                        x_in[0:msz, mt, n0:n0 + 292])
            return x_out

        for b in range(bl):
            x1 = resid.tile([128, 2, E], F32, name=f"x1_{b}", tag="x1")
            nc.sync.dma_start(x1[:, 0, :], idx_d[b, 0:128, :])
            nc.sync.dma_start(x1[0:72, 1, :], idx_d[b, 128:200, :])
            mem_t = work.tile([128, 2, E], BF16, name=f"mem_{b}", tag="memtok")
            nc.sync.dma_start(mem_t[:, 0, :], mem_d[b, 0:128, :])
            nc.sync.dma_start(mem_t[0:72, 1, :], mem_d[b, 128:200, :])

            # ---- self attention ----
            h1 = layernorm(x1, f"ln1_{b}")
            h1f = to_fm(h1, f"h1f_{b}")
            q1 = proj_heads(w_sb["wq_sa"], h1f, f"q1_{b}")
            k1 = proj_heads(w_sb["wk_sa"], h1f, f"k1_{b}")
            v1 = proj_tok(w_sb["wv_sa"], h1f, f"v1_{b}")
            x2 = attention(q1, k1, v1, wp_sa_sb, x1, True, f"sa_{b}", "x2")

            # ---- cross attention (k from memory, q/v from x2's LN) ----
            h2 = layernorm(x2, f"ln2_{b}")
            h2f = to_fm(h2, f"h2f_{b}")
            memf = to_fm(mem_t, f"memf_{b}")
            q2 = proj_heads(w_sb["wq_ca"], h2f, f"q2_{b}")
            k2 = proj_heads(w_sb["wk_ca"], memf, f"k2_{b}")
            v2 = proj_tok(w_sb["wv_ca"], h2f, f"v2_{b}")
            x3 = attention(q2, k2, v2, wp_ca_sb, x2, False, f"ca_{b}", "x3")

            # ---- FFN ----
            h3 = layernorm(x3, f"ln3_{b}")
            h3f = to_fm(h3, f"h3f_{b}")
            ff = work.tile([128, FFK, T], BF16, name=f"ff_{b}", tag="ff", bufs=1)
            for m in range(FFK):
                msz = FFB[m]
                ps = ps_mm.tile([128, T], F32, name=f"f1_{b}_{m}", tag="mm")
                for k in range(EK):
                    ksz = EB[k]
                    nc.tensor.matmul(
                        ps[0:msz, :], w1_sb[0:ksz, k, m * 128:m * 128 + msz],
                        h3f[0:ksz, k, :], start=(k == 0), stop=(k == EK - 1))
                nc.scalar.activation(
                    ff[0:msz, m, :], ps[0:msz, :],
                    mybir.ActivationFunctionType.Relu, bias=b1_sb[0:msz, m:m + 1])
            xo = resid.tile([128, 2, E], F32, name=f"xo_{b}", tag="xo")
            for mt, msz in TS:
                for n0 in NSPL:
                    ps = ps_mm.tile([128, 292], F32, name=f"f2_{b}_{mt}_{n0}", tag="mm")
                    for k in range(FFK):
                        ksz = FFB[k]
                        nc.tensor.matmul(
                            ps[0:msz, :], ff[0:ksz, k, mt * 128:mt * 128 + msz],
                            w2_sb[0:ksz, k, n0:n0 + 292],
                            start=(k == 0), stop=(k == FFK - 1))
                    nc.vector.tensor_add(
                        xo[0:msz, mt, n0:n0 + 292], ps[0:msz, :],
                        x3[0:msz, mt, n0:n0 + 292])
            nc.sync.dma_start(out_d[b, 0:128, :], xo[:, 0, :])
            nc.sync.dma_start(out_d[b, 128:200, :], xo[0:72, 1, :])

    nc.compile()
    return nc


def _pack_kxm(w, kblocks):
    """[K, M] fp32 -> [128, nk, M] bf16 with K zero-padded to 128*nk."""
    K, M = w.shape
    nk = len(kblocks)
    pad = np.zeros((128 * nk, M), np.float32)
    pad[:K] = w
    return np.ascontiguousarray(
        pad.reshape(nk, 128, M).transpose(1, 0, 2)).astype(BF16NP)


def prepare_inputs(inputs):
    """Host-side prep: LN folding, weight packing, per-core sharding.
    Returns list of 8 in_maps."""
    f = {k: np.asarray(v, np.float32) for k, v in inputs.items()}

    def fold(lnw, lnb, w3):
        # w3: [H, E, HS] per-head projection.  LN(x; w, b) @ W =
        # xhat @ (diag(w) W) + b W.  The additive term must be zero
        # (checked) because we don't apply qkv biases on-chip.
        wf = w3 * lnw[None, :, None]
        bias = np.einsum("e,hed->hd", lnb, w3) if lnb.any() else 0.0
        assert np.allclose(bias, 0.0, atol=1e-12), "nonzero folded qkv bias unsupported"
        return wf

    sa_q = fold(f["ln1_w"], f["ln1_b"], f["sa_q"])
    sa_k = fold(f["ln1_w"], f["ln1_b"], f["sa_k"])
    sa_v = fold(f["ln1_w"], f["ln1_b"], f["sa_v"])
    ca_q = fold(f["ln2_w"], f["ln2_b"], f["ca_q"])
    ca_v = fold(f["ln2_w"], f["ln2_b"], f["ca_v"])
    ca_k = f["ca_k"]  # cross-attn keys come from raw memory (no LN)
    w1 = f["ff_w1"] * f["ln3_w"][:, None]
    b1 = f["ff_b1"] + f["ln3_b"] @ f["ff_w1"]
    assert np.allclose(f["sa_pb"], 0.0) and np.allclose(f["ca_pb"], 0.0), \
        "nonzero attn proj bias unsupported"
    assert np.allclose(f["ff_b2"], 0.0), "nonzero ff_b2 unsupported"

    def stack_heads(w3):  # [H, E, HS] -> [E, H*HS]
        return np.ascontiguousarray(w3.transpose(1, 0, 2)).reshape(E, E)

    def pack_proj(pw):  # [E, E] -> [128(73 used), H, E] per-head K layout
        r = pw.reshape(H, HS, E)
        out = np.zeros((H, 128, E), np.float32)
        out[:, :HS, :] = r
        return np.ascontiguousarray(out.transpose(1, 0, 2)).astype(BF16NP)

    shared = {
        "wq_sa": _pack_kxm(stack_heads(sa_q), EB),
        "wk_sa": _pack_kxm(stack_heads(sa_k), EB),
        "wv_sa": _pack_kxm(stack_heads(sa_v), EB),
        "wq_ca": _pack_kxm(stack_heads(ca_q), EB),
        "wk_ca": _pack_kxm(stack_heads(ca_k), EB),
        "wv_ca": _pack_kxm(stack_heads(ca_v), EB),
        "wp_sa": pack_proj(f["sa_pw"]),
        "wp_ca": pack_proj(f["ca_pw"]),
        "w1": _pack_kxm(w1, EB),
        "w2": _pack_kxm(f["ff_w2"], FFB),
        "b1": np.ascontiguousarray(
            np.pad(b1, (0, 128 * FFK - FF)).reshape(FFK, 128).T),
        "ident": np.eye(128, dtype=BF16NP),
        "mask": np.triu(np.ones((128, 128), BF16NP)),
    }
    idx = f["idx"]
    mem = f["memory"].astype(BF16NP)
    in_maps = []
    for c in range(NCORES):
        m = dict(shared)
        m["idx"] = np.ascontiguousarray(idx[c * BL:(c + 1) * BL])
        m["mem"] = np.ascontiguousarray(mem[c * BL:(c + 1) * BL])
        in_maps.append(m)
    return in_maps


_NC_CACHE = {}


def kernel(**inputs):
    if BL not in _NC_CACHE:
        _NC_CACHE[BL] = build_nc(BL)
    nc = _NC_CACHE[BL]
    in_maps = prepare_inputs(inputs)
    res = run_bass_kernel_spmd(nc, in_maps, list(range(NCORES)))
    return np.concatenate([res.results[c]["out"] for c in range(NCORES)], axis=0)


# revision 12
# speedup vs baseline: 28.3449x; 28.3449x over previous
"""Trainium2 Bass kernel for nn_Decoder (dense transformer decoder layer).

Strategy: pure data-parallel over batch B=256 across 8 NeuronCores (32
batches/core).  Each core runs the full decoder layer on its shard; no
collectives.

Per-core dataflow (per batch sample b, T=200 tokens, E=584 features):
  - residual stream x kept TOKEN-major ([token_partitions, features]) in fp32
  - LayerNorm computed natively token-major (stats along free dim)
  - LN output h transposed on the PE (via identity matmul) to FEATURE-major
    bf16 [E, T] which feeds all weight matmuls (contraction dim = partitions)
  - attention scores computed TRANSPOSED: S^T[s,t] = k_h^T-slices @ q_h
    (lhsT=k, rhs=q), exp on the scalar engine (scale=E^-0.5 fused), softmax
    denominator Z via ones-vector matmul, 1/Z applied at the AV-output evict
    using a DMA partition-broadcast of the reciprocal row
  - AV output per head o_h [HS=73, T] feature-major; attn projection
    accumulates over heads (K-pieces of 73) and produces TOKEN-major output
    (lhsT = o_h with free dim = tokens) so the residual add is native
  - FFN: w1 feature-major out (Relu+bias fused at evict), w2 token-major out
  - all matmul operands bf16 (fp32 PSUM accumulation); residuals fp32

LayerNorm weights/biases are folded on the host into the adjacent projection
weights (exact for ln_w; ln_b folds require the resulting qkv/ffn biases to be
zero, which holds for this problem's inputs and is asserted).
"""

import sys

sys.path.insert(0, "/opt/trn_rl_repo")

from contextlib import ExitStack

import numpy as np
import ml_dtypes

import concourse.bass as bass
import concourse.bacc as bacc
import concourse.mybir as mybir
import concourse.tile as tile
from concourse.bass_utils import run_bass_kernel_spmd

F32 = mybir.dt.float32
BF16 = mybir.dt.bfloat16
BF16NP = ml_dtypes.bfloat16

B, T, E, H = 256, 200, 584, 8
HS = E // H  # 73
FF = 4 * E  # 2336
NCORES = 8
BL = B // NCORES  # 32
SCALE = float(E) ** -0.5
EPS = 1e-5

# tile decompositions
TS = [(0, 128), (1, 72)]  # token tiles (T=200)
EB = [128, 128, 128, 128, 72]  # E=584 partition blocks
EK = 5
FFB = [128] * 18 + [32]  # FF=2336 partition blocks
FFK = 19
NSPL = [0, 292]  # free-dim split of an E-sized matmul output (<=512 psum)


def _head_pieces(h):
    """Split head h's feature rows [73h, 73h+73) at 128-partition block
    boundaries -> list of (block, r0, r1)."""
    g0, g1 = HS * h, HS * h + HS
    out = []
    g = g0
    while g < g1:
        kb, r0 = divmod(g, 128)
        r1 = min(128, r0 + (g1 - g))
        out.append((kb, r0, r1))
        g += r1 - r0
    return out


def build_nc(bl=BL):
    """Build the single-core Bass program processing `bl` batch samples."""
    nc = bacc.Bacc(None, target_bir_lowering=False, debug=False)

    idx_d = nc.dram_tensor("idx", [bl, T, E], F32, kind="ExternalInput")
    mem_d = nc.dram_tensor("mem", [bl, T, E], BF16, kind="ExternalInput")
    w_names = ["wq_sa", "wk_sa", "wv_sa", "wq_ca", "wk_ca", "wv_ca"]
    w_d = {n: nc.dram_tensor(n, [128, EK, E], BF16, kind="ExternalInput") for n in w_names}
    wp_sa_d = nc.dram_tensor("wp_sa", [128, H, E], BF16, kind="ExternalInput")
    wp_ca_d = nc.dram_tensor("wp_ca", [128, H, E], BF16, kind="ExternalInput")
    w1_d = nc.dram_tensor("w1", [128, EK, FF], BF16, kind="ExternalInput")
    w2_d = nc.dram_tensor("w2", [128, FFK, E], BF16, kind="ExternalInput")
    b1_d = nc.dram_tensor("b1", [128, FFK], F32, kind="ExternalInput")
    ident_d = nc.dram_tensor("ident", [128, 128], BF16, kind="ExternalInput")
    mask_d = nc.dram_tensor("mask", [128, 128], BF16, kind="ExternalInput")
    out_d = nc.dram_tensor("out", [bl, T, E], F32, kind="ExternalOutput")

    with tile.TileContext(nc) as tc, ExitStack() as ctx:
        wpool = ctx.enter_context(tc.tile_pool(name="wpool", bufs=1))
        w_sb = {}
        for n in w_names:
            w_sb[n] = wpool.tile([128, EK, E], BF16, name=n + "_sb")
            nc.sync.dma_start(w_sb[n][:], w_d[n][:])
        wp_sa_sb = wpool.tile([128, H, E], BF16, name="wp_sa_sb")
        nc.sync.dma_start(wp_sa_sb[:], wp_sa_d[:])
        wp_ca_sb = wpool.tile([128, H, E], BF16, name="wp_ca_sb")
        nc.sync.dma_start(wp_ca_sb[:], wp_ca_d[:])
        w1_sb = wpool.tile([128, EK, FF], BF16, name="w1_sb")
        nc.sync.dma_start(w1_sb[:], w1_d[:])
        w2_sb = wpool.tile([128, FFK, E], BF16, name="w2_sb")
        nc.sync.dma_start(w2_sb[:], w2_d[:])
        b1_sb = wpool.tile([128, FFK], F32, name="b1_sb")
        nc.sync.dma_start(b1_sb[:], b1_d[:])
        ident_sb = wpool.tile([128, 128], BF16, name="ident_sb")
        nc.sync.dma_start(ident_sb[:], ident_d[:])
        mask_sb = wpool.tile([128, 128], BF16, name="mask_sb")
        nc.sync.dma_start(mask_sb[:], mask_d[:])
        ones_sb = wpool.tile([128, 1], BF16, name="ones_sb")
        nc.vector.memset(ones_sb[:], 1.0)
        eps_sb = wpool.tile([128, 1], F32, name="eps_sb")
        nc.vector.memset(eps_sb[:], EPS)

        resid = ctx.enter_context(tc.tile_pool(name="resid", bufs=2))
        work = ctx.enter_context(tc.tile_pool(name="work", bufs=2))
        stat = ctx.enter_context(tc.tile_pool(name="stat", bufs=2))
        scr = ctx.enter_context(tc.tile_pool(name="scr", bufs=2))
        opool = ctx.enter_context(tc.tile_pool(name="opool", bufs=2))
        ps_mm = ctx.enter_context(tc.tile_pool(name="ps_mm", bufs=3, space="PSUM"))
        ps_s = ctx.enter_context(tc.tile_pool(name="ps_s", bufs=2, space="PSUM"))
        ps_z = ctx.enter_context(tc.tile_pool(name="ps_z", bufs=1, space="PSUM"))
        ps_o = ctx.enter_context(tc.tile_pool(name="ps_o", bufs=1, space="PSUM"))
        ps_tp = ctx.enter_context(tc.tile_pool(name="ps_tp", bufs=1, space="PSUM"))
        dpool = ctx.enter_context(tc.tile_pool(name="dpool", bufs=4, space="DRAM"))

        def layernorm(x_t, name):
            """x_t [128,2,E] f32 -> h_tok [128,2,E] bf16, normalized (no w/b)."""
            h_tok = work.tile([128, 2, E], BF16, name=name, tag="htok")
            for tt, tsz in TS:
                xs = x_t[0:tsz, tt, :]
                s1 = stat.tile([128, 1], F32, name=name + f"_s1_{tt}", tag="s1")
                s2 = stat.tile([128, 1], F32, name=name + f"_s2_{tt}", tag="s2")
                sc1 = scr.tile([128, E], F32, name=name + f"_sc1_{tt}", tag="sc1", bufs=1)
                sc2 = scr.tile([128, E], F32, name=name + f"_sc2_{tt}", tag="sc2", bufs=1)
                nc.scalar.activation(
                    sc1[0:tsz, :], xs, mybir.ActivationFunctionType.Copy,
                    accum_out=s1[0:tsz, :])
                nc.scalar.activation(
                    sc2[0:tsz, :], xs, mybir.ActivationFunctionType.Square,
                    accum_out=s2[0:tsz, :])
                negm = stat.tile([128, 1], F32, name=name + f"_nm_{tt}", tag="nm")
                nc.scalar.mul(negm[0:tsz, :], s1[0:tsz, :], -1.0 / E)
                m2 = stat.tile([128, 1], F32, name=name + f"_m2_{tt}", tag="m2")
                nc.vector.tensor_mul(m2[0:tsz, :], negm[0:tsz, :], negm[0:tsz, :])
                ss = stat.tile([128, 1], F32, name=name + f"_ss_{tt}", tag="ss")
                nc.scalar.mul(ss[0:tsz, :], s2[0:tsz, :], 1.0 / E)
                var = stat.tile([128, 1], F32, name=name + f"_var_{tt}", tag="var")
                nc.vector.tensor_sub(var[0:tsz, :], ss[0:tsz, :], m2[0:tsz, :])
                sd = stat.tile([128, 1], F32, name=name + f"_sd_{tt}", tag="sd")
                nc.scalar.activation(
                    sd[0:tsz, :], var[0:tsz, :],
                    mybir.ActivationFunctionType.Sqrt, bias=eps_sb[0:tsz, :])
                r = stat.tile([128, 1], F32, name=name + f"_r_{tt}", tag="r")
                nc.vector.reciprocal(r[0:tsz, :], sd[0:tsz, :])
                nmr = stat.tile([128, 1], F32, name=name + f"_nmr_{tt}", tag="nmr")
                nc.vector.tensor_mul(nmr[0:tsz, :], negm[0:tsz, :], r[0:tsz, :])
                nc.scalar.activation(
                    h_tok[0:tsz, tt, :], xs,
                    mybir.ActivationFunctionType.Identity,
                    bias=nmr[0:tsz, :], scale=r[0:tsz, :])
            return h_tok

        def to_fm(src_tok, name):
            """[128,2,E] bf16 token-major -> [128,EK,T] bf16 feature-major."""
            dst = work.tile([128, EK, T], BF16, name=name, tag="hfm")
            for tt, tsz in TS:
                for eb in range(EK):
                    esz = EB[eb]
                    ps = ps_tp.tile([128, 128], BF16, name=f"{name}_tp", tag="tp")
                    nc.tensor.transpose(
                        ps[0:esz, 0:tsz],
                        src_tok[0:tsz, tt, eb * 128:eb * 128 + esz],
                        ident_sb[0:tsz, 0:tsz])
                    nc.any.tensor_copy(
                        dst[0:esz, eb, tt * 128:tt * 128 + tsz], ps[0:esz, 0:tsz])
            return dst

        def proj_heads(w, src_fm, name):
            """q/k projection, per-head aligned output [HS, H, T] bf16
            (matmul operands must start at partition 0/32/64, so each head's
            73 rows get their own partition-base-0 slot)."""
            dst = work.tile([HS, H, T], BF16, name=name, tag=name[:1])
            for h in range(H):
                ps = ps_mm.tile([128, T], F32, name=f"{name}_ps", tag="mm")
                for k in range(EK):
                    ksz = EB[k]
                    nc.tensor.matmul(
                        ps[0:HS, :], w[0:ksz, k, HS * h:HS * h + HS],
                        src_fm[0:ksz, k, :], start=(k == 0), stop=(k == EK - 1))
                nc.any.tensor_copy(dst[:, h, :], ps[0:HS, :])
            return dst

        def proj_tok(w, src_fm, name):
            """v projection: token-major out [128,2,E] bf16."""
            dst = work.tile([128, 2, E], BF16, name=name, tag="vtok")
            for mt, msz in TS:
                for n0 in NSPL:
                    ps = ps_mm.tile([128, 292], F32, name=f"{name}_ps", tag="mm")
                    for k in range(EK):
                        ksz = EB[k]
                        nc.tensor.matmul(
                            ps[0:msz, :],
                            src_fm[0:ksz, k, mt * 128:mt * 128 + msz],
                            w[0:ksz, k, n0:n0 + 292],
                            start=(k == 0), stop=(k == EK - 1))
                    nc.any.tensor_copy(dst[0:msz, mt, n0:n0 + 292], ps[0:msz, :])
            return dst

        def attention(q_fm, k_fm, v_tok, wp_sb, x_in, causal, name, xtag):
            """Full MHA given feature-major q/k, token-major v.  Returns new
            residual (token-major f32): x_out = proj(attn) + x_in."""
            o_sb = []
            for h in range(H):
                expS = opool.tile([128, 2, T], BF16, name=f"{name}_e{h}", tag="expS")
                # scores S^T: s-tile 0 (s=0:128), all t (causal: t>=s region)
                ps0 = ps_s.tile([128, T], F32, name=f"{name}_s0_{h}", tag="s")
                nc.tensor.matmul(
                    ps0[0:128, :], k_fm[:, h, 0:128], q_fm[:, h, :],
                    start=True, stop=True)
                nc.scalar.activation(
                    expS[0:128, 0, :], ps0[0:128, :],
                    mybir.ActivationFunctionType.Exp, scale=SCALE)
                if causal:
                    nc.vector.tensor_mul(
                        expS[0:128, 0, 0:128], expS[0:128, 0, 0:128],
                        mask_sb[0:128, 0:128])
                # s-tile 1 (s=128:200)
                ps1 = ps_s.tile([128, T], F32, name=f"{name}_s1_{h}", tag="s")
                t0 = 128 if causal else 0
                tsz1 = T - t0
                nc.tensor.matmul(
                    ps1[0:72, 0:tsz1], k_fm[:, h, 128:200], q_fm[:, h, t0:T],
                    start=True, stop=True)
                nc.scalar.activation(
                    expS[0:72, 1, t0:T], ps1[0:72, 0:tsz1],
                    mybir.ActivationFunctionType.Exp, scale=SCALE)
                if causal:
                    nc.vector.tensor_mul(
                        expS[0:72, 1, 128:200], expS[0:72, 1, 128:200],
                        mask_sb[0:72, 0:72])
                # softmax denominator Z = column-sum over s
                zps = ps_z.tile([1, T], F32, name=f"{name}_z{h}", tag="z")
                if causal:
                    nc.tensor.matmul(zps[0:1, :], ones_sb[0:128, 0:1],
                                     expS[0:128, 0, :], start=True, stop=False)
                    nc.tensor.matmul(zps[0:1, 128:200], ones_sb[0:72, 0:1],
                                     expS[0:72, 1, 128:200], start=False, stop=True)
                else:
                    nc.tensor.matmul(zps[0:1, :], ones_sb[0:128, 0:1],
                                     expS[0:128, 0, :], start=True, stop=False)
                    nc.tensor.matmul(zps[0:1, :], ones_sb[0:72, 0:1],
                                     expS[0:72, 1, :], start=False, stop=True)
                zr = stat.tile([1, T], F32, name=f"{name}_zr{h}", tag="zr")
                nc.vector.reciprocal(zr[0:1, :], zps[0:1, :])
                zb = scr.tile([128, T], F32, name=f"{name}_zb{h}", tag="zb")
                nc.gpsimd.partition_broadcast(zb[:, :], zr[0:1, :])
                # attention-weighted values: o_h [HS, T] feature-major
                hs0 = HS * h
                ops = ps_o.tile([HS, T], F32, name=f"{name}_o{h}", tag="o")
                if causal:
                    nc.tensor.matmul(ops[:, 0:128], v_tok[0:128, 0, hs0:hs0 + HS],
                                     expS[0:128, 0, 0:128], start=True, stop=True)
                    nc.tensor.matmul(ops[:, 128:200], v_tok[0:128, 0, hs0:hs0 + HS],
                                     expS[0:128, 0, 128:200], start=True, stop=False)
                    nc.tensor.matmul(ops[:, 128:200], v_tok[0:72, 1, hs0:hs0 + HS],
                                     expS[0:72, 1, 128:200], start=False, stop=True)
                else:
                    nc.tensor.matmul(ops[:, :], v_tok[0:128, 0, hs0:hs0 + HS],
                                     expS[0:128, 0, :], start=True, stop=False)
                    nc.tensor.matmul(ops[:, :], v_tok[0:72, 1, hs0:hs0 + HS],
                                     expS[0:72, 1, :], start=False, stop=True)
                osb = opool.tile([HS, T], BF16, name=f"{name}_ob{h}", tag=f"o{h}")
                nc.vector.tensor_mul(osb[:, :], ops[:, :], zb[0:HS, :])
                o_sb.append(osb)
            # projection (accumulate over heads) + residual, token-major out
            x_out = resid.tile([128, 2, E], F32, name=f"{name}_xo", tag=xtag)
            for mt, msz in TS:
                for n0 in NSPL:
                    ps = ps_mm.tile([128, 292], F32, name=f"{name}_pj", tag="mm")
                    for h in range(H):
                        nc.tensor.matmul(
                            ps[0:msz, :],
                            o_sb[h][:, mt * 128:mt * 128 + msz],
                            wp_sb[0:HS, h, n0:n0 + 292],
                            start=(h == 0), stop=(h == H - 1))
                    nc.vector.tensor_add(
                        x_out[0:msz, mt, n0:n0 + 292], ps[0:msz, :],
                        x_in[0:msz, mt, n0:n0 + 292])
            return x_out

        for b in range(bl):
            x1 = resid.tile([128, 2, E], F32, name=f"x1_{b}", tag="x1")
            nc.sync.dma_start(x1[:, 0, :], idx_d[b, 0:128, :])
            nc.sync.dma_start(x1[0:72, 1, :], idx_d[b, 128:200, :])
            mem_t = work.tile([128, 2, E], BF16, name=f"mem_{b}", tag="memtok")
            nc.sync.dma_start(mem_t[:, 0, :], mem_d[b, 0:128, :])
            nc.sync.dma_start(mem_t[0:72, 1, :], mem_d[b, 128:200, :])

            # ---- self attention ----
            h1 = layernorm(x1, f"ln1_{b}")
            h1f = to_fm(h1, f"h1f_{b}")
            q1 = proj_heads(w_sb["wq_sa"], h1f, f"q1_{b}")
            k1 = proj_heads(w_sb["wk_sa"], h1f, f"k1_{b}")
            v1 = proj_tok(w_sb["wv_sa"], h1f, f"v1_{b}")
            x2 = attention(q1, k1, v1, wp_sa_sb, x1, True, f"sa_{b}", "x2")

            # ---- cross attention (k from memory, q/v from x2's LN) ----
            h2 = layernorm(x2, f"ln2_{b}")
            h2f = to_fm(h2, f"h2f_{b}")
            memf = to_fm(mem_t, f"memf_{b}")
            q2 = proj_heads(w_sb["wq_ca"], h2f, f"q2_{b}")
            k2 = proj_heads(w_sb["wk_ca"], memf, f"k2_{b}")
            v2 = proj_tok(w_sb["wv_ca"], h2f, f"v2_{b}")
            x3 = attention(q2, k2, v2, wp_ca_sb, x2, False, f"ca_{b}", "x3")

            # ---- FFN ----
            h3 = layernorm(x3, f"ln3_{b}")
            h3f = to_fm(h3, f"h3f_{b}")
            ff = work.tile([128, FFK, T], BF16, name=f"ff_{b}", tag="ff", bufs=1)
            for m in range(FFK):
                msz = FFB[m]
                ps = ps_mm.tile([128, T], F32, name=f"f1_{b}_{m}", tag="mm")
                for k in range(EK):
                    ksz = EB[k]
                    nc.tensor.matmul(
                        ps[0:msz, :], w1_sb[0:ksz, k, m * 128:m * 128 + msz],
                        h3f[0:ksz, k, :], start=(k == 0), stop=(k == EK - 1))
                nc.scalar.activation(
                    ff[0:msz, m, :], ps[0:msz, :],
                    mybir.ActivationFunctionType.Relu, bias=b1_sb[0:msz, m:m + 1])
            xo = resid.tile([128, 2, E], F32, name=f"xo_{b}", tag="xo")
            for mt, msz in TS:
                for n0 in NSPL:
                    ps = ps_mm.tile([128, 292], F32, name=f"f2_{b}_{mt}_{n0}", tag="mm")
                    for k in range(FFK):
                        ksz = FFB[k]
                        nc.tensor.matmul(
                            ps[0:msz, :], ff[0:ksz, k, mt * 128:mt * 128 + msz],
                            w2_sb[0:ksz, k, n0:n0 + 292],
                            start=(k == 0), stop=(k == FFK - 1))
                    nc.vector.tensor_add(
                        xo[0:msz, mt, n0:n0 + 292], ps[0:msz, :],
                        x3[0:msz, mt, n0:n0 + 292])
            nc.sync.dma_start(out_d[b, 0:128, :], xo[:, 0, :])
            nc.sync.dma_start(out_d[b, 128:200, :], xo[0:72, 1, :])

    nc.compile()
    return nc


def _pack_kxm(w, kblocks):
    """[K, M] fp32 -> [128, nk, M] bf16 with K zero-padded to 128*nk."""
    K, M = w.shape
    nk = len(kblocks)
    pad = np.zeros((128 * nk, M), np.float32)
    pad[:K] = w
    return np.ascontiguousarray(
        pad.reshape(nk, 128, M).transpose(1, 0, 2)).astype(BF16NP)


def prepare_inputs(inputs):
    """Host-side prep: LN folding, weight packing, per-core sharding.
    Returns list of 8 in_maps."""
    f = {k: np.asarray(v, np.float32) for k, v in inputs.items()}

    def fold(lnw, lnb, w3):
        # w3: [H, E, HS] per-head projection.  LN(x; w, b) @ W =
        # xhat @ (diag(w) W) + b W.  The additive term must be zero
        # (checked) because we don't apply qkv biases on-chip.
        wf = w3 * lnw[None, :, None]
        bias = np.einsum("e,hed->hd", lnb, w3) if lnb.any() else 0.0
        assert np.allclose(bias, 0.0, atol=1e-12), "nonzero folded qkv bias unsupported"
        return wf

    sa_q = fold(f["ln1_w"], f["ln1_b"], f["sa_q"])
    sa_k = fold(f["ln1_w"], f["ln1_b"], f["sa_k"])
    sa_v = fold(f["ln1_w"], f["ln1_b"], f["sa_v"])
    ca_q = fold(f["ln2_w"], f["ln2_b"], f["ca_q"])
    ca_v = fold(f["ln2_w"], f["ln2_b"], f["ca_v"])
    ca_k = f["ca_k"]  # cross-attn keys come from raw memory (no LN)
    w1 = f["ff_w1"] * f["ln3_w"][:, None]
    b1 = f["ff_b1"] + f["ln3_b"] @ f["ff_w1"]
    assert np.allclose(f["sa_pb"], 0.0) and np.allclose(f["ca_pb"], 0.0), \
        "nonzero attn proj bias unsupported"
    assert np.allclose(f["ff_b2"], 0.0), "nonzero ff_b2 unsupported"

    def stack_heads(w3):  # [H, E, HS] -> [E, H*HS]
        return np.ascontiguousarray(w3.transpose(1, 0, 2)).reshape(E, E)

    def pack_proj(pw):  # [E, E] -> [128(73 used), H, E] per-head K layout
        r = pw.reshape(H, HS, E)
        out = np.zeros((H, 128, E), np.float32)
        out[:, :HS, :] = r
        return np.ascontiguousarray(out.transpose(1, 0, 2)).astype(BF16NP)

    shared = {
        "wq_sa": _pack_kxm(stack_heads(sa_q), EB),
        "wk_sa": _pack_kxm(stack_heads(sa_k), EB),
        "wv_sa": _pack_kxm(stack_heads(sa_v), EB),
        "wq_ca": _pack_kxm(stack_heads(ca_q), EB),
        "wk_ca": _pack_kxm(stack_heads(ca_k), EB),
        "wv_ca": _pack_kxm(stack_heads(ca_v), EB),
        "wp_sa": pack_proj(f["sa_pw"]),
        "wp_ca": pack_proj(f["ca_pw"]),
        "w1": _pack_kxm(w1, EB),
        "w2": _pack_kxm(f["ff_w2"], FFB),
        "b1": np.ascontiguousarray(
            np.pad(b1, (0, 128 * FFK - FF)).reshape(FFK, 128).T),
        "ident": np.eye(128, dtype=BF16NP),
        "mask": np.triu(np.ones((128, 128), BF16NP)),
    }
    idx = f["idx"]
    mem = f["memory"].astype(BF16NP)
    in_maps = []
    for c in range(NCORES):
        m = dict(shared)
        m["idx"] = np.ascontiguousarray(idx[c * BL:(c + 1) * BL])
        m["mem"] = np.ascontiguousarray(mem[c * BL:(c + 1) * BL])
        in_maps.append(m)
    return in_maps


_NC_CACHE = {}


def kernel(**inputs):
    if BL not in _NC_CACHE:
        _NC_CACHE[BL] = build_nc(BL)
    nc = _NC_CACHE[BL]
    in_maps = prepare_inputs(inputs)
    res = run_bass_kernel_spmd(nc, in_maps, list(range(NCORES)))
    return np.concatenate([res.results[c]["out"] for c in range(NCORES)], axis=0)
